# revision 70
# baseline (speedup 1.0000x reference)
"""Trainium2 Bass kernel for nn_ConditionPooler (ragged cross-attention pooler).

Per core (data-parallel over B=16 scenes, S=2 scenes/core on 8 cores):
  scores[n,(h,t)] = feat @ A          A[c,(h,t)] = sum_d qh[t,h,d] w_k[h*DH+d,c]
  P = exp(scores)   (no max-subtract: scores ~ N(0,1); k-bias cancels in softmax)
  U[(h,t),c]  = sum_n P[n,(h,t)] feat[n,c]     (pad rows: P=1, feat=0 -> 0)
  den[(h,t)]  = sum_n P[n,(h,t)] - npad        (pad rows contribute exp(0)=1)
  Uhat = U/den; attnT = wv' Uhat^T; out = attn @ w_o^T (+folded biases)
  h = out + query; z = LN(h); ff = gelu(z@(w1*g)^T + b1') @ w2^T + b2; out = h+ff

Layout/perf notes:
 - feat is uploaded twice (both bf16): n-major tiles (U matmul rhs) and
   c-major pre-transposed tiles (scores lhsT), so the PE never transposes in
   the stream.  Chunked DMAs rotate over the sync/ACT/gpsimd queues; the
   first chunks ramp (1,2,2,4 tiles) so the PE starts ~1.5us in.
 - streaming is software-pipelined 2 tiles deep: tile i's U/den matmuls are
   issued between tile i+2's scores halves, hiding the exp (ACT) latency.
 - PSUM: scores/den rotate in one 4-buf bank pool; per-scene U accumulators
   take 4 banks (2 scenes x 2 halves).  8 banks total; the epilogue reuses
   freed banks via pool rotation.
 - epilogue: scene-0's Uhat/UT overlap scene-1 streaming; the rest is batched
   across scenes.  LN uses bn_stats + 1/sqrt = exp(-0.5 ln) to stay in the
   ln+exp ACT table (no sqrt table switch); the gelu table load is prewarmed
   behind a data-pinned dummy so it hides under the zT/ff1 matmuls.  ff1
   computes its output transposed ([hid, st]) so no transposes are needed
   between gelu and ff2; ff2/residual/store run in column halves to overlap.
"""

import numpy as np

C = 512
T = 32
H = 8
DH = C // H
NCORES = 8
HT = H * T  # 256

NJ_UP = 4   # featT c-blocks uploaded (of 4); rest transposed on PE
CHMAX = 6   # max tiles per DMA chunk

_CACHE = {}


def _apply_tile_patch():
    """This walrus build allows only one sem wait on CTRL-encoded (Drain)
    instructions; TileContext's tail drain carries the whole global clock.
    Split the extra waits onto standalone sync-engine nops."""
    import concourse.tile as tile_mod
    import concourse.mybir as mybir
    from concourse.vector_clock import ScopedClock

    if getattr(tile_mod.TileContext, "_drain_patched", False):
        return

    def _patched(self, tick_clock, wait_clock):
        nc = self.nc
        drain_inst = nc.sync.drain()
        wait_clock.add_sem_waits(
            drain_inst.ins, ScopedClock({None: tick_clock.global_clock})
        )
        si = drain_inst.ins.sync_info
        if si is not None and si.on_wait is not None and len(si.on_wait) > 1:
            waits = list(si.on_wait)
            si.on_wait = waits[:1]
            for w in waits[1:]:
                nop = nc.sync.nop(nofuse=True)
                nsi = nop.ins.sync_info
                if nsi is None:
                    nop.ins.sync_info = mybir.SyncInfo(on_wait=[w], on_update=[])
                else:
                    nsi.on_wait = [w]
        nc.all_engine_barrier()
        assert self.sems is not None
        popped = nc._tile_sem_poison_stack.pop()
        assert popped is self._sem_poison
        nc.clear_and_free_semaphores(list(self.sems.allocated().values()))
        nc.all_engine_barrier()

    tile_mod.TileContext._drain_and_barrier = _patched
    tile_mod.TileContext._drain_patched = True


def _split_multi_waits(nc):
    """This walrus build caps sync waits at 1 per instruction (2 for
    EventSemaphore). Tile emits several on some instructions; hoist the
    extras onto same-engine NoOps inserted just before."""
    import concourse.mybir as mybir

    cnt = [0]
    for f in nc.m.functions:
        for b in f.blocks:
            newlist = []
            for inst in b.instructions:
                si = inst.sync_info
                if si is not None and si.on_wait is not None and len(si.on_wait) > 1:
                    waits = list(si.on_wait)
                    for w in waits[:-1]:
                        cnt[0] += 1
                        nop = mybir.InstNoOp(
                            name=f"I-wsplit-{cnt[0]}", ins=[], outs=[]
                        )
                        nop.engine = inst.engine
                        nop.sync_info = mybir.SyncInfo(on_wait=[w], on_update=[])
                        newlist.append(nop)
                    si.on_wait = waits[-1:]
                newlist.append(inst)
            b.instructions = newlist
    return nc


def _chunk_plan(NT, ramp):
    """Split NT tiles into DMA chunks of <=CHMAX tiles.  The first scene ramps
    up with small chunks so the PE starts within ~1.5us of kernel entry."""
    sizes = []
    left = NT
    if ramp:
        for c in (1, 2, 4, 4):
            if left - c < 0:
                break
            sizes.append(c)
            left -= c
    while left > 0:
        c = min(CHMAX, left)
        # avoid a tiny trailing chunk: balance the last two
        if left - c == 1 and c > 2:
            c -= 1
        sizes.append(c)
        left -= c
    return sizes


def _build(NT, S, nj_up=NJ_UP, split=True):
    import concourse.bass as bass
    import concourse.mybir as mybir
    import concourse.tile as tile

    _apply_tile_patch()

    f32 = mybir.dt.float32
    bf16 = mybir.dt.bfloat16
    AF = mybir.ActivationFunctionType
    ALU = mybir.AluOpType
    ST = S * T
    ntr = 4 - nj_up  # c-blocks transposed on PE per tile

    chunks = [_chunk_plan(NT, ramp=(s == 0)) for s in range(S)]

    nc = bass.Bass()
    featp = nc.dram_tensor("featp", [S * NT * 128, C], bf16, kind="ExternalInput")
    if nj_up:
        featTp = nc.dram_tensor(
            "featTp", [S * NT * 128 * nj_up, 128], bf16, kind="ExternalInput"
        )
    akT_d = nc.dram_tensor("akT", [C, HT], bf16, kind="ExternalInput")
    wvT_d = nc.dram_tensor("wvT", [C, C], bf16, kind="ExternalInput")
    woT_d = nc.dram_tensor("woT", [C, C], bf16, kind="ExternalInput")
    w1gT_d = nc.dram_tensor("w1gT", [C, 2 * C], bf16, kind="ExternalInput")
    b1e_d = nc.dram_tensor("b1e", [1, 2 * C], bf16, kind="ExternalInput")
    w2T_d = nc.dram_tensor("w2T", [2 * C, C], bf16, kind="ExternalInput")
    b2e_d = nc.dram_tensor("b2e", [1, C], bf16, kind="ExternalInput")
    qb_d = nc.dram_tensor("qb", [T, C], bf16, kind="ExternalInput")
    idb_d = nc.dram_tensor("identb", [128, 128], bf16, kind="ExternalInput")
    npad_d = nc.dram_tensor("npad", [128, S], f32, kind="ExternalInput")
    outp = nc.dram_tensor("outp", [ST, C], f32, kind="ExternalOutput")

    with tile.TileContext(nc) as tc:
        with tc.tile_pool(name="const", bufs=1) as const:
            akT = const.tile([128, 4, HT], bf16, tag="akT")
            nc.sync.dma_start(akT[:], akT_d.rearrange("(j p) f -> p j f", p=128))
            identb = const.tile([128, 128], bf16, tag="identb")
            npadT = const.tile([128, S], f32, tag="npadT")
            qb2 = const.tile([ST, C], bf16, tag="qb2")
            wvT = const.tile([128, 4, C], bf16, tag="wvT")
            woT = const.tile([128, 4, C], bf16, tag="woT")
            w1gT = const.tile([128, 4, 2 * C], bf16, tag="w1gT")
            w2T = const.tile([128, 8, C], bf16, tag="w2T")
            b1e = const.tile([1, 2 * C], bf16, tag="b1e")
            b2e = const.tile([1, C], bf16, tag="b2e")

            def load_consts_early():
                g = nc.gpsimd
                g.dma_start(identb[:], idb_d[:])
                g.dma_start(npadT[:], npad_d[:])
                for s in range(S):
                    g.dma_start(qb2[s * T : (s + 1) * T, :], qb_d[:])

            def load_epi_weights(dep=None):
                g = nc.gpsimd
                if dep is not None:
                    # token write with a true data dep: pins the (otherwise
                    # dependency-free) weight DMAs behind the early stream so
                    # the scheduler can't hoist them into the DMA ramp
                    nc.gpsimd.tensor_copy(wvT[0:1, 0:1, 0:1], dep)
                g.dma_start(wvT[:], wvT_d.rearrange("(j p) f -> p j f", p=128))
                g.dma_start(woT[:], woT_d.rearrange("(j p) f -> p j f", p=128))
                g.dma_start(w1gT[:], w1gT_d.rearrange("(j p) f -> p j f", p=128))
                g.dma_start(w2T[:], w2T_d.rearrange("(j p) f -> p j f", p=128))
                g.dma_start(b1e[:], b1e_d[:])
                g.dma_start(b2e[:], b2e_d[:])

            onescol = const.tile([128, 1], bf16, tag="onescol")
            nc.vector.memset(onescol[:], 1.0)
            # prewarm the exp ACT table while the first feat chunks load —
            # otherwise the first streamed exp pays the ~1.4us table load
            dummye = const.tile([1, 1], f32, tag="dummye")
            nc.scalar.activation(dummye[:], onescol[0:1, 0:1], AF.Exp)
            ones64 = const.tile([1, ST], bf16, tag="ones64")
            nc.vector.memset(ones64[:], 1.0)
            epsc = const.tile([ST, 1], f32, tag="epsc")
            nc.vector.memset(epsc[:], 1e-5)

            with tc.tile_pool(name="epi", bufs=1) as epi:
                dacc = epi.tile([128, 2, S], f32, tag="dacc")
                rden = epi.tile([128, 2, S], f32, tag="rden")
                # separate tiles per h2 so the ACT and DVE evacuations don't
                # serialize on a same-tile write dependency
                Uhat0 = epi.tile([128, S, C], bf16, tag="Uhat0")
                Uhat1 = epi.tile([128, S, C], bf16, tag="Uhat1")
                UT = epi.tile([128, 4, S, HT], bf16, tag="UT")

                with (
                    tc.tile_pool(name="psw", bufs=4, space="PSUM") as psw,
                    tc.tile_pool(name="psU", bufs=2, space="PSUM") as psU,
                    tc.tile_pool(name="fb", bufs=4) as fpool,
                    tc.tile_pool(name="ftb", bufs=4) as ftpool,
                    tc.tile_pool(name="sb", bufs=4) as spool,
                ):
                    def uhat_ut(s, Upair):
                        """den -> 1/den, Uhat = U/den, UT = Uhat^T (to SBUF).
                        The two scaled evacuations run on ACT and DVE in
                        parallel."""
                        # den = max(sum(P) - npad, tiny): the clamp keeps an
                        # empty scene from producing inf/NaN downstream
                        nc.vector.tensor_scalar(
                            rden[:, :, s], dacc[:, :, s], npadT[:, s : s + 1],
                            1e-30, op0=ALU.subtract, op1=ALU.max,
                        )
                        nc.vector.reciprocal(rden[:, :, s], rden[:, :, s])
                        nc.scalar.activation(
                            Uhat0[:, s, :], Upair[0][:], AF.Copy,
                            scale=rden[:, 0, s : s + 1],
                        )
                        nc.vector.tensor_scalar_mul(
                            Uhat1[:, s, :], Upair[1][:], rden[:, 1, s : s + 1]
                        )
                        # per-h2 transpose + copy in separate PSUM banks so
                        # the h2=1 transposes don't contend with the h2=0
                        # copy's bank read; the first half's attn matmuls
                        # start while the second half drains
                        for h2, Uh, tg in ((0, Uhat0, "Ua"), (1, Uhat1, "Ub")):
                            utp = psU.tile(
                                [128, HT], f32, tag=tg, name=f"utp{s}{h2}"
                            ).bitcast(bf16)  # [128, 512]
                            for jc in range(4):
                                nc.tensor.transpose(
                                    utp[:, jc * 128 : (jc + 1) * 128],
                                    Uh[:, s, jc * 128 : (jc + 1) * 128],
                                    identb[:],
                                )
                            nc.vector.tensor_copy(
                                UT[:, :, s, h2 * 128 : (h2 + 1) * 128],
                                utp.rearrange("p (j n) -> p j n", j=4),
                            )

                    def emit_U(st):
                        """Drain the pended tile: U accumulation + den."""
                        PT, F, s, i, Upair = st
                        ps_d = psw.tile([128, 2], f32, tag="w", name="ps_d")
                        for h2 in range(2):
                            nc.tensor.matmul(
                                Upair[h2][:],
                                PT[:, h2 * 128 : (h2 + 1) * 128],
                                F,
                                start=(i == 0), stop=(i == NT - 1),
                            )
                            nc.tensor.matmul(
                                ps_d[:, h2 : h2 + 1],
                                PT[:, h2 * 128 : (h2 + 1) * 128],
                                onescol[:],
                                start=True, stop=True,
                            )
                        if i == 0:
                            nc.vector.tensor_copy(dacc[:, :, s], ps_d[:])
                        else:
                            nc.vector.tensor_add(
                                dacc[:, :, s], dacc[:, :, s], ps_d[:]
                            )

                    # software-pipelined streaming (2 tiles deep): tile i's
                    # U/den matmuls are issued between tile i+2's two scores
                    # halves, hiding the exp (ACT) and FTr-copy (DVE)
                    # latencies behind a full tile of PE work.
                    pend = []
                    pend_uhat = None
                    for s in range(S):
                        Upair = (
                            psU.tile([128, C], f32, tag="Ua", name=f"Ua{s}"),
                            psU.tile([128, C], f32, tag="Ub", name=f"Ub{s}"),
                        )
                        i = 0
                        toff = s * NT
                        for ci, cg in enumerate(chunks[s]):
                            r0 = (toff + i) * 128
                            Fc = fpool.tile([128, CHMAX, C], bf16, tag="F")
                            nc.sync.dma_start(
                                Fc[:, :cg, :],
                                featp[r0 : r0 + 128 * cg, :].rearrange(
                                    "(p i) c -> p i c", p=128
                                ),
                            )
                            if nj_up:
                                rT0 = (toff + i) * 128 * nj_up
                                FTc = ftpool.tile(
                                    [128, CHMAX, nj_up, 128], bf16, tag="FT"
                                )
                                # keep FTc off the ACT queue entirely — its
                                # engine-hold would delay the exps.  gpsimd
                                # carries half of scene 0 (it is free until
                                # the weight loads start at scene 1).
                                if s == 0:
                                    # ramp chunks all on gpsimd (sync is busy
                                    # with akT + the first Fc transfers)
                                    ftq = (
                                        nc.gpsimd
                                        if ci <= 2
                                        else (nc.gpsimd, nc.sync)[ci % 2]
                                    )
                                else:
                                    ftq = nc.sync
                                ftq.dma_start(
                                    FTc[:, :cg, :, :],
                                    featTp[rT0 : rT0 + 128 * cg * nj_up, :].rearrange(
                                        "(p i j) n -> p i j n", p=128, i=cg
                                    ),
                                )
                            if s == 0 and ci == 2:
                                load_consts_early()
                            if s == 1 and ci == 0:
                                load_epi_weights(dep=dacc[0:1, 0:1, 0:1])
                            if s == 1 and ci == 1 and pend_uhat is not None:
                                uhat_ut(*pend_uhat)
                                pend_uhat = None
                            for ii in range(cg):
                                F = Fc[:, ii, :]
                                if ntr:
                                    ps_tr = psw.tile(
                                        [128, 256], f32, tag="w", name="ps_tr"
                                    ).bitcast(bf16)  # [128, 512]
                                    for j in range(nj_up, 4):
                                        nc.tensor.transpose(
                                            ps_tr[:, (j - nj_up) * 128 : (j - nj_up + 1) * 128],
                                            F[:, j * 128 : (j + 1) * 128],
                                            identb[:],
                                        )
                                    FTr = spool.tile([128, ntr * 128], bf16, tag="FTr")
                                    nc.vector.tensor_copy(
                                        FTr[:], ps_tr[:, : ntr * 128]
                                    )
                                ps_s = psw.tile([128, HT], f32, tag="w", name="ps_s")
                                for j in range(4):
                                    lhs = (
                                        FTc[:, ii, j, :]
                                        if j < nj_up
                                        else FTr[:, (j - nj_up) * 128 : (j - nj_up + 1) * 128]
                                    )
                                    nc.tensor.matmul(
                                        ps_s[:], lhs, akT[:, j, :],
                                        start=(j == 0), stop=(j == 3),
                                    )
                                    if j == 1 and len(pend) >= 2:
                                        emit_U(pend.pop(0))
                                PT = spool.tile([128, HT], bf16, tag="PT")
                                for h2 in range(2):  # halves: the pipeline
                                    # drain's U matmuls start after half-exp
                                    nc.scalar.activation(
                                        PT[:, h2 * 128 : (h2 + 1) * 128],
                                        ps_s[:, h2 * 128 : (h2 + 1) * 128],
                                        AF.Exp,
                                    )
                                pend.append((PT, F, s, i, Upair))
                                i += 1
                        pend_uhat = (s, Upair)
                    for st in pend:
                        emit_U(st)
                    pend = []
                    # last scene's Uhat/UT (still inside streaming pools:
                    # uses the psU rotation for the transpose bank)
                    uhat_ut(*pend_uhat)

                # ---- batched tail (streaming PSUM banks now free) ----
                with (
                    tc.tile_pool(name="pst", bufs=1, space="PSUM") as pst,
                    tc.tile_pool(name="psz", bufs=1, space="PSUM") as psz,
                ):
                    # attention value projection: at_ps[(hh dh), gq, (s t)]
                    at_ps = pst.tile([128, 4, ST], f32, tag="at")
                    for gq in range(4):
                        for jc in range(4):
                            for hh in range(2):
                                h = 2 * gq + hh
                                nc.tensor.matmul(
                                    at_ps[hh * 64 : (hh + 1) * 64, gq, :],
                                    wvT[:, jc, h * DH : (h + 1) * DH],
                                    UT[:, jc, :, h * T : (h + 1) * T],
                                    start=(jc == 0), stop=(jc == 3),
                                )
                    # per-gq evacuation so the first woT matmul starts while
                    # the later copies are still draining
                    at_sb = epi.tile([128, 4, ST], bf16, tag="at_sb")
                    for gq in range(4):
                        nc.scalar.activation(
                            at_sb[:, gq, :], at_ps[:, gq, :], AF.Copy
                        )

                    # output projection -> h; the residual (query + folded
                    # biases, identical rows per scene) is injected into the
                    # same PSUM accumulation via an identity matmul.
                    ph = psz.tile([ST, C], f32, tag="ph")
                    nc.tensor.matmul(
                        ph[:], identb[:ST, :ST], qb2[:], start=True, stop=False
                    )
                    for gq in range(4):
                        nc.tensor.matmul(
                            ph[:], at_sb[:, gq, :], woT[:, gq, :],
                            start=False, stop=(gq == 3),
                        )

                    # layernorm via bn_stats; 1/sqrt(var+eps) = exp(-0.5 ln())
                    # keeps the ACT table in the ln+exp set (no sqrt switch)
                    st6 = epi.tile([ST, 6], f32, tag="st6")
                    nc.vector.bn_stats(st6[:], ph[:])
                    mv = epi.tile([ST, 2], f32, tag="mv")
                    nc.vector.bn_aggr(mv[:], st6[:])
                    lv = epi.tile([ST, 1], f32, tag="lv")
                    nc.scalar.activation(lv[:], mv[:, 1:2], AF.Ln, bias=epsc[:])
                    rstd = epi.tile([ST, 1], f32, tag="rstd")
                    nc.scalar.activation(rstd[:], lv[:], AF.Exp, scale=-0.5)
                    z = epi.tile([ST, C], bf16, tag="z")
                    nc.vector.tensor_scalar(
                        z[:], ph[:], mv[:, 0:1], rstd[:],
                        op0=ALU.subtract, op1=ALU.mult,
                    )
                    # prewarm the gelu table under the zT/ff1 PE work (the z
                    # input dep pins it after the last LN exp on the ACT
                    # queue — earlier would evict the ln+exp table mid-LN)
                    dummy = epi.tile([1, 1], f32, tag="dummy")
                    nc.scalar.activation(dummy[:], rstd[0:1, 0:1], AF.Gelu)

                    # zT
                    zps = pst.tile([128, 4, ST], bf16, tag="zps")
                    for jc in range(4):
                        nc.tensor.transpose(
                            zps[:, jc, :], z[:, jc * 128 : (jc + 1) * 128],
                            identb[:ST, :ST],
                        )
                    zT = epi.tile([128, 4, ST], bf16, tag="zT")
                    nc.vector.tensor_copy(zT[:], zps[:])
                    # evacuate h for the final residual AFTER the zT copy:
                    # the token write makes h_sb depend on zT, else the
                    # greedy scheduler runs this 658ns copy first and delays
                    # ff1.  (fin may not read two PSUM operands at once.)
                    h_sb = epi.tile([ST, C], f32, tag="h_sb")
                    nc.vector.tensor_copy(h_sb[0:1, 0:1], zT[0:1, 0:1, 0:1])
                    nc.vector.tensor_copy(h_sb[:], ph[:])

                    # ff1 (transposed output: [hid, st]); the bias matmuls
                    # have no data deps and fill the LN-chain PE bubble
                    pf = psz.tile([128, 8, ST], f32, tag="pf")
                    for hb in range(8):
                        nc.tensor.matmul(
                            pf[:, hb, :],
                            b1e[:, hb * 128 : (hb + 1) * 128],
                            ones64[:],
                            start=(hb == 0), stop=False,
                            skip_group_check=True,
                        )
                    NQ = 2
                    QW = C // NQ
                    po = [
                        pst.tile([ST, QW], f32, tag=f"po{h}", name=f"po{h}")
                        for h in range(NQ)
                    ]
                    for q in range(NQ):
                        nc.tensor.matmul(
                            po[q][:],
                            ones64[:],
                            b2e[:, q * QW : (q + 1) * QW],
                            start=True, stop=False,
                            skip_group_check=True,
                        )
                    for hb in range(8):
                        for jc in range(4):
                            nc.tensor.matmul(
                                pf[:, hb, :],
                                w1gT[:, jc, hb * 128 : (hb + 1) * 128],
                                zT[:, jc, :],
                                start=False, stop=(jc == 3 and hb == 7),
                                skip_group_check=True,
                            )
                    gmT = epi.tile([128, 8, ST], bf16, tag="gmT")
                    nc.scalar.activation(gmT[:, 0:4, :], pf[:, 0:4, :], AF.Gelu)
                    nc.scalar.activation(gmT[:, 4:8, :], pf[:, 4:8, :], AF.Gelu)

                    # ff2 + residual, in column halves so the first half's
                    # add+store overlaps the second half's matmuls
                    fin = epi.tile([ST, C], f32, tag="fin")
                    for q in range(NQ):
                        cs = slice(q * QW, (q + 1) * QW)
                        for k in range(8):
                            nc.tensor.matmul(
                                po[q][:], gmT[:, k, :], w2T[:, k, cs],
                                start=False, stop=(k == 7),
                                skip_group_check=True,
                            )
                        nc.vector.tensor_add(fin[:, cs], h_sb[:, cs], po[q][:])
                        nc.sync.dma_start(outp[:, cs], fin[:, cs])

    if split:
        _split_multi_waits(nc)
    return nc


def _host_prep(inputs, nj_up=NJ_UP):
    import ml_dtypes

    bf = ml_dtypes.bfloat16
    feat = np.asarray(inputs["feat"], dtype=np.float32)
    batch_idx = np.asarray(inputs["batch_idx"]).astype(np.int64)
    B = int(np.asarray(inputs["batch_size"]))
    query = np.asarray(inputs["query"], dtype=np.float32)
    g_q = np.asarray(inputs["g_q"], np.float32)
    b_q = np.asarray(inputs["b_q"], np.float32)
    w_q = np.asarray(inputs["w_q"], np.float32)
    w_k = np.asarray(inputs["w_k"], np.float32)
    w_v = np.asarray(inputs["w_v"], np.float32)
    b_q_in = np.asarray(inputs["b_q_in"], np.float32)
    b_v_in = np.asarray(inputs["b_v_in"], np.float32)
    w_o = np.asarray(inputs["w_o"], np.float32)
    b_o = np.asarray(inputs["b_o"], np.float32)
    g_ff = np.asarray(inputs["g_ff"], np.float32)
    b_ff = np.asarray(inputs["b_ff"], np.float32)
    w1 = np.asarray(inputs["w1"], np.float32)
    b1 = np.asarray(inputs["b1"], np.float32)
    w2 = np.asarray(inputs["w2"], np.float32)
    b2 = np.asarray(inputs["b2"], np.float32)

    S = B // NCORES
    counts = np.bincount(batch_idx, minlength=B)
    offs = np.concatenate([[0], np.cumsum(counts)])
    NT = max(1, int(np.ceil(counts.max() / 128)))
    P = NT * 128

    # padded per-scene feat in bf16, [B, P, C]
    featpad = np.zeros((B, P, C), dtype=bf)
    for b in range(B):
        featpad[b, : counts[b]] = feat[offs[b] : offs[b + 1]].astype(bf)

    # n-major tiles: per tile [128, C] with rows p = n-within-tile; host
    # layout groups rows as (tile, p) -> flat (S*NT*128, C)
    # SBUF chunk [p, i, c] <- rows (p*cg + i) of the chunk block, so build
    # [tile, 128, C] then per-chunk transpose later.  DMA reads contiguous
    # rows; we lay the whole buffer chunk-by-chunk in [p, i, c] order.
    chunk_lists = [_chunk_plan(NT, ramp=(s == 0)) for s in range(S)]

    featA = np.empty((NCORES, S * NT * 128, C), dtype=bf)
    featTA = (
        np.empty((NCORES, S * NT * 128 * nj_up, 128), dtype=bf) if nj_up else None
    )
    npad = np.empty((NCORES, 128, S), dtype=np.float32)
    for c in range(NCORES):
        for s in range(S):
            b = c * S + s
            npad[c, :, s] = P - counts[b]
            tiles = featpad[b].reshape(NT, 128, C)
            i = 0
            for cg in chunk_lists[s]:
                blk = tiles[i : i + cg]  # [cg, 128, C]
                r0 = (s * NT + i) * 128
                featA[c, r0 : r0 + 128 * cg] = (
                    blk.transpose(1, 0, 2).reshape(128 * cg, C)
                )
                if nj_up:
                    # [p, i, j, n] where channel = j*128 + p
                    tb = blk.reshape(cg, 128, 4, 128)[:, :, :nj_up, :]
                    # tb[i, n, j, p] -> [p, i, j, n]
                    tT = np.ascontiguousarray(tb.transpose(3, 0, 2, 1))
                    rT0 = (s * NT + i) * 128 * nj_up
                    featTA[c, rT0 : rT0 + 128 * cg * nj_up] = tT.reshape(
                        128 * cg * nj_up, 128
                    )
                i += cg

    # query-side fold (host; tiny)
    q = query[0]
    mu = q.mean(-1, keepdims=True)
    var = ((q - mu) ** 2).mean(-1, keepdims=True)
    qn = (q - mu) / np.sqrt(var + 1e-5) * g_q + b_q
    qh = (qn @ w_q.T + b_q_in) / np.sqrt(DH)  # [T, C]
    A = np.einsum(
        "thd,hdc->cht", qh.reshape(T, H, DH), w_k.reshape(H, DH, C)
    ).reshape(C, H * T)

    consts = dict(
        akT=np.ascontiguousarray(A.astype(bf)),
        wvT=np.ascontiguousarray(w_v.T.astype(bf)),
        woT=np.ascontiguousarray(w_o.T.astype(bf)),
        w1gT=np.ascontiguousarray((w1 * g_ff[None, :]).T.astype(bf)),
        b1e=(b1 + w1 @ b_ff).reshape(1, 2 * C).astype(bf),
        w2T=np.ascontiguousarray(w2.T.astype(bf)),
        b2e=b2.reshape(1, C).astype(bf),
        qb=np.ascontiguousarray(query[0] + (b_o + w_o @ b_v_in)[None, :]).astype(
            bf
        ),
        identb=np.eye(128, dtype=bf),
    )
    in_maps = []
    for c in range(NCORES):
        m = dict(consts)
        m["featp"] = featA[c]
        if nj_up:
            m["featTp"] = featTA[c]
        m["npad"] = npad[c]
        in_maps.append(m)
    return in_maps, NT, S, B


def kernel(**inputs):
    from concourse.bass_utils import run_bass_kernel_spmd

    in_maps, NT, S, B = _host_prep(inputs, NJ_UP)
    key = (NT, S, NJ_UP)
    if key not in _CACHE:
        _CACHE[key] = _build(NT, S, nj_up=NJ_UP)
    nc = _CACHE[key]
    res = run_bass_kernel_spmd(nc, in_maps, core_ids=list(range(NCORES)))
    out = np.empty((B, T, C), dtype=np.float32)
    for c in range(NCORES):
        o = res.results[c]["outp"]
        for s in range(S):
            out[c * S + s] = o[s * T : (s + 1) * T]
    return out


# revision 71
# speedup vs baseline: 1.0015x; 1.0015x over previous
"""Trainium2 Bass kernel for nn_ConditionPooler (ragged cross-attention pooler).

Per core (data-parallel over B=16 scenes, S=2 scenes/core on 8 cores):
  scores[n,(h,t)] = feat @ A          A[c,(h,t)] = sum_d qh[t,h,d] w_k[h*DH+d,c]
  P = exp(scores)   (no max-subtract: scores ~ N(0,1); k-bias cancels in softmax)
  U[(h,t),c]  = sum_n P[n,(h,t)] feat[n,c]     (pad rows: P=1, feat=0 -> 0)
  den[(h,t)]  = sum_n P[n,(h,t)] - npad        (pad rows contribute exp(0)=1)
  Uhat = U/den; attnT = wv' Uhat^T; out = attn @ w_o^T (+folded biases)
  h = out + query; z = LN(h); ff = gelu(z@(w1*g)^T + b1') @ w2^T + b2; out = h+ff

Layout/perf notes:
 - feat is uploaded twice (both bf16): n-major tiles (U matmul rhs) and
   c-major pre-transposed tiles (scores lhsT), so the PE never transposes in
   the stream.  Chunked DMAs rotate over the sync/ACT/gpsimd queues; the
   first chunks ramp (1,2,2,4 tiles) so the PE starts ~1.5us in.
 - streaming is software-pipelined 2 tiles deep: tile i's U/den matmuls are
   issued between tile i+2's scores halves, hiding the exp (ACT) latency.
 - PSUM: scores/den rotate in one 4-buf bank pool; per-scene U accumulators
   take 4 banks (2 scenes x 2 halves).  8 banks total; the epilogue reuses
   freed banks via pool rotation.
 - epilogue: scene-0's Uhat/UT overlap scene-1 streaming; the rest is batched
   across scenes.  LN uses bn_stats + 1/sqrt = exp(-0.5 ln) to stay in the
   ln+exp ACT table (no sqrt table switch); the gelu table load is prewarmed
   behind a data-pinned dummy so it hides under the zT/ff1 matmuls.  ff1
   computes its output transposed ([hid, st]) so no transposes are needed
   between gelu and ff2; ff2/residual/store run in column halves to overlap.
"""

import numpy as np

C = 512
T = 32
H = 8
DH = C // H
NCORES = 8
HT = H * T  # 256

NJ_UP = 4   # featT c-blocks uploaded (of 4); rest transposed on PE
CHMAX = 6   # max tiles per DMA chunk

_CACHE = {}


def _apply_tile_patch():
    """This walrus build allows only one sem wait on CTRL-encoded (Drain)
    instructions; TileContext's tail drain carries the whole global clock.
    Split the extra waits onto standalone sync-engine nops."""
    import concourse.tile as tile_mod
    import concourse.mybir as mybir
    from concourse.vector_clock import ScopedClock

    if getattr(tile_mod.TileContext, "_drain_patched", False):
        return

    def _patched(self, tick_clock, wait_clock):
        nc = self.nc
        drain_inst = nc.sync.drain()
        wait_clock.add_sem_waits(
            drain_inst.ins, ScopedClock({None: tick_clock.global_clock})
        )
        si = drain_inst.ins.sync_info
        if si is not None and si.on_wait is not None and len(si.on_wait) > 1:
            waits = list(si.on_wait)
            si.on_wait = waits[:1]
            for w in waits[1:]:
                nop = nc.sync.nop(nofuse=True)
                nsi = nop.ins.sync_info
                if nsi is None:
                    nop.ins.sync_info = mybir.SyncInfo(on_wait=[w], on_update=[])
                else:
                    nsi.on_wait = [w]
        nc.all_engine_barrier()
        assert self.sems is not None
        popped = nc._tile_sem_poison_stack.pop()
        assert popped is self._sem_poison
        nc.clear_and_free_semaphores(list(self.sems.allocated().values()))
        nc.all_engine_barrier()

    tile_mod.TileContext._drain_and_barrier = _patched
    tile_mod.TileContext._drain_patched = True


def _split_multi_waits(nc):
    """This walrus build caps sync waits at 1 per instruction (2 for
    EventSemaphore). Tile emits several on some instructions; hoist the
    extras onto same-engine NoOps inserted just before."""
    import concourse.mybir as mybir

    cnt = [0]
    for f in nc.m.functions:
        for b in f.blocks:
            newlist = []
            for inst in b.instructions:
                si = inst.sync_info
                if si is not None and si.on_wait is not None and len(si.on_wait) > 1:
                    waits = list(si.on_wait)
                    for w in waits[:-1]:
                        cnt[0] += 1
                        nop = mybir.InstNoOp(
                            name=f"I-wsplit-{cnt[0]}", ins=[], outs=[]
                        )
                        nop.engine = inst.engine
                        nop.sync_info = mybir.SyncInfo(on_wait=[w], on_update=[])
                        newlist.append(nop)
                    si.on_wait = waits[-1:]
                newlist.append(inst)
            b.instructions = newlist
    return nc


def _chunk_plan(NT, ramp):
    """Split NT tiles into DMA chunks of <=CHMAX tiles.  The first scene ramps
    up with small chunks so the PE starts within ~1.5us of kernel entry."""
    sizes = []
    left = NT
    if ramp:
        for c in (1, 2, 4, 4):
            if left - c < 0:
                break
            sizes.append(c)
            left -= c
    while left > 0:
        c = min(CHMAX, left)
        # avoid a tiny trailing chunk: balance the last two
        if left - c == 1 and c > 2:
            c -= 1
        sizes.append(c)
        left -= c
    return sizes


def _build(NT, S, nj_up=NJ_UP, split=True):
    import concourse.bass as bass
    import concourse.mybir as mybir
    import concourse.tile as tile

    _apply_tile_patch()

    f32 = mybir.dt.float32
    bf16 = mybir.dt.bfloat16
    AF = mybir.ActivationFunctionType
    ALU = mybir.AluOpType
    ST = S * T
    ntr = 4 - nj_up  # c-blocks transposed on PE per tile

    chunks = [_chunk_plan(NT, ramp=(s == 0)) for s in range(S)]

    nc = bass.Bass()
    featp = nc.dram_tensor("featp", [S * NT * 128, C], bf16, kind="ExternalInput")
    if nj_up:
        featTp = nc.dram_tensor(
            "featTp", [S * NT * 128 * nj_up, 128], bf16, kind="ExternalInput"
        )
    akT_d = nc.dram_tensor("akT", [C, HT], bf16, kind="ExternalInput")
    wvT_d = nc.dram_tensor("wvT", [C, C], bf16, kind="ExternalInput")
    woT_d = nc.dram_tensor("woT", [C, C], bf16, kind="ExternalInput")
    w1gT_d = nc.dram_tensor("w1gT", [C, 2 * C], bf16, kind="ExternalInput")
    b1e_d = nc.dram_tensor("b1e", [1, 2 * C], bf16, kind="ExternalInput")
    w2T_d = nc.dram_tensor("w2T", [2 * C, C], bf16, kind="ExternalInput")
    b2e_d = nc.dram_tensor("b2e", [1, C], bf16, kind="ExternalInput")
    qb_d = nc.dram_tensor("qb", [T, C], bf16, kind="ExternalInput")
    idb_d = nc.dram_tensor("identb", [128, 128], bf16, kind="ExternalInput")
    npad_d = nc.dram_tensor("npad", [128, S], f32, kind="ExternalInput")
    outp = nc.dram_tensor("outp", [ST, C], f32, kind="ExternalOutput")

    with tile.TileContext(nc) as tc:
        with tc.tile_pool(name="const", bufs=1) as const:
            akT = const.tile([128, 4, HT], bf16, tag="akT")
            nc.sync.dma_start(akT[:], akT_d.rearrange("(j p) f -> p j f", p=128))
            identb = const.tile([128, 128], bf16, tag="identb")
            npadT = const.tile([128, S], f32, tag="npadT")
            qb2 = const.tile([ST, C], bf16, tag="qb2")
            wvT = const.tile([128, 4, C], bf16, tag="wvT")
            woT = const.tile([128, 4, C], bf16, tag="woT")
            w1gT = const.tile([128, 4, 2 * C], bf16, tag="w1gT")
            w2T = const.tile([128, 8, C], bf16, tag="w2T")
            b1e = const.tile([1, 2 * C], bf16, tag="b1e")
            b2e = const.tile([1, C], bf16, tag="b2e")

            def load_consts_early():
                g = nc.gpsimd
                g.dma_start(identb[:], idb_d[:])
                g.dma_start(npadT[:], npad_d[:])
                for s in range(S):
                    g.dma_start(qb2[s * T : (s + 1) * T, :], qb_d[:])

            def load_epi_weights(dep=None):
                g = nc.gpsimd
                if dep is not None:
                    # token write with a true data dep: pins the (otherwise
                    # dependency-free) weight DMAs behind the early stream so
                    # the scheduler can't hoist them into the DMA ramp
                    nc.gpsimd.tensor_copy(wvT[0:1, 0:1, 0:1], dep)
                g.dma_start(wvT[:], wvT_d.rearrange("(j p) f -> p j f", p=128))
                g.dma_start(woT[:], woT_d.rearrange("(j p) f -> p j f", p=128))
                g.dma_start(w1gT[:], w1gT_d.rearrange("(j p) f -> p j f", p=128))
                g.dma_start(w2T[:], w2T_d.rearrange("(j p) f -> p j f", p=128))
                g.dma_start(b1e[:], b1e_d[:])
                g.dma_start(b2e[:], b2e_d[:])

            onescol = const.tile([128, 1], bf16, tag="onescol")
            nc.vector.memset(onescol[:], 1.0)
            # prewarm the exp ACT table while the first feat chunks load —
            # otherwise the first streamed exp pays the ~1.4us table load
            dummye = const.tile([1, 1], f32, tag="dummye")
            nc.scalar.activation(dummye[:], onescol[0:1, 0:1], AF.Exp)
            ones64 = const.tile([1, ST], bf16, tag="ones64")
            nc.vector.memset(ones64[:], 1.0)
            epsc = const.tile([ST, 1], f32, tag="epsc")
            nc.vector.memset(epsc[:], 1e-5)

            with tc.tile_pool(name="epi", bufs=1) as epi:
                dacc = epi.tile([128, 2, S], f32, tag="dacc")
                rden = epi.tile([128, 2, S], f32, tag="rden")
                # separate tiles per h2 so the ACT and DVE evacuations don't
                # serialize on a same-tile write dependency
                Uhat0 = epi.tile([128, S, C], bf16, tag="Uhat0")
                Uhat1 = epi.tile([128, S, C], bf16, tag="Uhat1")
                UT = epi.tile([128, 4, S, HT], bf16, tag="UT")

                with (
                    tc.tile_pool(name="psw", bufs=4, space="PSUM") as psw,
                    tc.tile_pool(name="psU", bufs=2, space="PSUM") as psU,
                    tc.tile_pool(name="fb", bufs=4) as fpool,
                    tc.tile_pool(name="ftb", bufs=4) as ftpool,
                    tc.tile_pool(name="sb", bufs=4) as spool,
                ):
                    def uhat_ut(s, Upair):
                        """den -> 1/den, Uhat = U/den, UT = Uhat^T (to SBUF).
                        The two scaled evacuations run on ACT and DVE in
                        parallel."""
                        # den = max(sum(P) - npad, tiny): the clamp keeps an
                        # empty scene from producing inf/NaN downstream
                        nc.vector.tensor_scalar(
                            rden[:, :, s], dacc[:, :, s], npadT[:, s : s + 1],
                            1e-30, op0=ALU.subtract, op1=ALU.max,
                        )
                        nc.vector.reciprocal(rden[:, :, s], rden[:, :, s])
                        nc.scalar.activation(
                            Uhat0[:, s, :], Upair[0][:], AF.Copy,
                            scale=rden[:, 0, s : s + 1],
                        )
                        nc.vector.tensor_scalar_mul(
                            Uhat1[:, s, :], Upair[1][:], rden[:, 1, s : s + 1]
                        )
                        # per-h2 transpose + copy in separate PSUM banks so
                        # the h2=1 transposes don't contend with the h2=0
                        # copy's bank read; the first half's attn matmuls
                        # start while the second half drains
                        for h2, Uh, tg in ((0, Uhat0, "Ua"), (1, Uhat1, "Ub")):
                            utp = psU.tile(
                                [128, HT], f32, tag=tg, name=f"utp{s}{h2}"
                            ).bitcast(bf16)  # [128, 512]
                            for jc in range(4):
                                nc.tensor.transpose(
                                    utp[:, jc * 128 : (jc + 1) * 128],
                                    Uh[:, s, jc * 128 : (jc + 1) * 128],
                                    identb[:],
                                )
                            nc.vector.tensor_copy(
                                UT[:, :, s, h2 * 128 : (h2 + 1) * 128],
                                utp.rearrange("p (j n) -> p j n", j=4),
                            )

                    def emit_U(st):
                        """Drain the pended tile: U accumulation + den."""
                        PT, F, s, i, Upair = st
                        ps_d = psw.tile([128, 2], f32, tag="w", name="ps_d")
                        for h2 in range(2):
                            nc.tensor.matmul(
                                Upair[h2][:],
                                PT[:, h2 * 128 : (h2 + 1) * 128],
                                F,
                                start=(i == 0), stop=(i == NT - 1),
                            )
                            nc.tensor.matmul(
                                ps_d[:, h2 : h2 + 1],
                                PT[:, h2 * 128 : (h2 + 1) * 128],
                                onescol[:],
                                start=True, stop=True,
                            )
                        if i == 0:
                            nc.vector.tensor_copy(dacc[:, :, s], ps_d[:])
                        else:
                            nc.vector.tensor_add(
                                dacc[:, :, s], dacc[:, :, s], ps_d[:]
                            )

                    # software-pipelined streaming (2 tiles deep): tile i's
                    # U/den matmuls are issued between tile i+2's two scores
                    # halves, hiding the exp (ACT) and FTr-copy (DVE)
                    # latencies behind a full tile of PE work.
                    pend = []
                    pend_uhat = None
                    for s in range(S):
                        Upair = (
                            psU.tile([128, C], f32, tag="Ua", name=f"Ua{s}"),
                            psU.tile([128, C], f32, tag="Ub", name=f"Ub{s}"),
                        )
                        i = 0
                        toff = s * NT
                        for ci, cg in enumerate(chunks[s]):
                            r0 = (toff + i) * 128
                            Fc = fpool.tile([128, CHMAX, C], bf16, tag="F")
                            nc.sync.dma_start(
                                Fc[:, :cg, :],
                                featp[r0 : r0 + 128 * cg, :].rearrange(
                                    "(p i) c -> p i c", p=128
                                ),
                            )
                            if nj_up:
                                rT0 = (toff + i) * 128 * nj_up
                                FTc = ftpool.tile(
                                    [128, CHMAX, nj_up, 128], bf16, tag="FT"
                                )
                                # keep FTc off the ACT queue entirely — its
                                # engine-hold would delay the exps.  gpsimd
                                # carries half of scene 0 (it is free until
                                # the weight loads start at scene 1).
                                if s == 0:
                                    # chunk 0 takes the fast HWDGE path on
                                    # ACT (idle at t=0); the rest of the ramp
                                    # rides gpsimd (sync is busy with akT +
                                    # the first Fc transfers)
                                    if ci == 0:
                                        ftq = nc.scalar
                                    elif ci <= 2:
                                        ftq = nc.gpsimd
                                    else:
                                        ftq = (nc.gpsimd, nc.sync)[ci % 2]
                                else:
                                    ftq = nc.sync
                                ftq.dma_start(
                                    FTc[:, :cg, :, :],
                                    featTp[rT0 : rT0 + 128 * cg * nj_up, :].rearrange(
                                        "(p i j) n -> p i j n", p=128, i=cg
                                    ),
                                )
                            if s == 0 and ci == 2:
                                load_consts_early()
                            if s == 1 and ci == 0:
                                load_epi_weights(dep=dacc[0:1, 0:1, 0:1])
                            if s == 1 and ci == 1 and pend_uhat is not None:
                                uhat_ut(*pend_uhat)
                                pend_uhat = None
                            for ii in range(cg):
                                F = Fc[:, ii, :]
                                if ntr:
                                    ps_tr = psw.tile(
                                        [128, 256], f32, tag="w", name="ps_tr"
                                    ).bitcast(bf16)  # [128, 512]
                                    for j in range(nj_up, 4):
                                        nc.tensor.transpose(
                                            ps_tr[:, (j - nj_up) * 128 : (j - nj_up + 1) * 128],
                                            F[:, j * 128 : (j + 1) * 128],
                                            identb[:],
                                        )
                                    FTr = spool.tile([128, ntr * 128], bf16, tag="FTr")
                                    nc.vector.tensor_copy(
                                        FTr[:], ps_tr[:, : ntr * 128]
                                    )
                                ps_s = psw.tile([128, HT], f32, tag="w", name="ps_s")
                                for j in range(4):
                                    lhs = (
                                        FTc[:, ii, j, :]
                                        if j < nj_up
                                        else FTr[:, (j - nj_up) * 128 : (j - nj_up + 1) * 128]
                                    )
                                    nc.tensor.matmul(
                                        ps_s[:], lhs, akT[:, j, :],
                                        start=(j == 0), stop=(j == 3),
                                    )
                                    if j == 1 and len(pend) >= 2:
                                        emit_U(pend.pop(0))
                                PT = spool.tile([128, HT], bf16, tag="PT")
                                for h2 in range(2):  # halves: the pipeline
                                    # drain's U matmuls start after half-exp
                                    nc.scalar.activation(
                                        PT[:, h2 * 128 : (h2 + 1) * 128],
                                        ps_s[:, h2 * 128 : (h2 + 1) * 128],
                                        AF.Exp,
                                    )
                                pend.append((PT, F, s, i, Upair))
                                i += 1
                        pend_uhat = (s, Upair)
                    for st in pend:
                        emit_U(st)
                    pend = []
                    # last scene's Uhat/UT (still inside streaming pools:
                    # uses the psU rotation for the transpose bank)
                    uhat_ut(*pend_uhat)

                # ---- batched tail (streaming PSUM banks now free) ----
                with (
                    tc.tile_pool(name="pst", bufs=1, space="PSUM") as pst,
                    tc.tile_pool(name="psz", bufs=1, space="PSUM") as psz,
                ):
                    # attention value projection: at_ps[(hh dh), gq, (s t)]
                    at_ps = pst.tile([128, 4, ST], f32, tag="at")
                    for gq in range(4):
                        for jc in range(4):
                            for hh in range(2):
                                h = 2 * gq + hh
                                nc.tensor.matmul(
                                    at_ps[hh * 64 : (hh + 1) * 64, gq, :],
                                    wvT[:, jc, h * DH : (h + 1) * DH],
                                    UT[:, jc, :, h * T : (h + 1) * T],
                                    start=(jc == 0), stop=(jc == 3),
                                )
                    # per-gq evacuation so the first woT matmul starts while
                    # the later copies are still draining
                    at_sb = epi.tile([128, 4, ST], bf16, tag="at_sb")
                    for gq in range(4):
                        nc.scalar.activation(
                            at_sb[:, gq, :], at_ps[:, gq, :], AF.Copy
                        )

                    # output projection -> h; the residual (query + folded
                    # biases, identical rows per scene) is injected into the
                    # same PSUM accumulation via an identity matmul.
                    ph = psz.tile([ST, C], f32, tag="ph")
                    nc.tensor.matmul(
                        ph[:], identb[:ST, :ST], qb2[:], start=True, stop=False
                    )
                    for gq in range(4):
                        nc.tensor.matmul(
                            ph[:], at_sb[:, gq, :], woT[:, gq, :],
                            start=False, stop=(gq == 3),
                        )

                    # layernorm via bn_stats; 1/sqrt(var+eps) = exp(-0.5 ln())
                    # keeps the ACT table in the ln+exp set (no sqrt switch)
                    st6 = epi.tile([ST, 6], f32, tag="st6")
                    nc.vector.bn_stats(st6[:], ph[:])
                    mv = epi.tile([ST, 2], f32, tag="mv")
                    nc.vector.bn_aggr(mv[:], st6[:])
                    lv = epi.tile([ST, 1], f32, tag="lv")
                    nc.scalar.activation(lv[:], mv[:, 1:2], AF.Ln, bias=epsc[:])
                    rstd = epi.tile([ST, 1], f32, tag="rstd")
                    nc.scalar.activation(rstd[:], lv[:], AF.Exp, scale=-0.5)
                    z = epi.tile([ST, C], bf16, tag="z")
                    nc.vector.tensor_scalar(
                        z[:], ph[:], mv[:, 0:1], rstd[:],
                        op0=ALU.subtract, op1=ALU.mult,
                    )
                    # prewarm the gelu table under the zT/ff1 PE work (the z
                    # input dep pins it after the last LN exp on the ACT
                    # queue — earlier would evict the ln+exp table mid-LN)
                    dummy = epi.tile([1, 1], f32, tag="dummy")
                    nc.scalar.activation(dummy[:], rstd[0:1, 0:1], AF.Gelu)

                    # zT
                    zps = pst.tile([128, 4, ST], bf16, tag="zps")
                    for jc in range(4):
                        nc.tensor.transpose(
                            zps[:, jc, :], z[:, jc * 128 : (jc + 1) * 128],
                            identb[:ST, :ST],
                        )
                    zT = epi.tile([128, 4, ST], bf16, tag="zT")
                    nc.vector.tensor_copy(zT[:], zps[:])
                    # evacuate h for the final residual AFTER the zT copy:
                    # the token write makes h_sb depend on zT, else the
                    # greedy scheduler runs this 658ns copy first and delays
                    # ff1.  (fin may not read two PSUM operands at once.)
                    h_sb = epi.tile([ST, C], f32, tag="h_sb")
                    nc.vector.tensor_copy(h_sb[0:1, 0:1], zT[0:1, 0:1, 0:1])
                    nc.vector.tensor_copy(h_sb[:], ph[:])

                    # ff1 (transposed output: [hid, st]); the bias matmuls
                    # have no data deps and fill the LN-chain PE bubble
                    pf = psz.tile([128, 8, ST], f32, tag="pf")
                    for hb in range(8):
                        nc.tensor.matmul(
                            pf[:, hb, :],
                            b1e[:, hb * 128 : (hb + 1) * 128],
                            ones64[:],
                            start=(hb == 0), stop=False,
                            skip_group_check=True,
                        )
                    NQ = 2
                    QW = C // NQ
                    po = [
                        pst.tile([ST, QW], f32, tag=f"po{h}", name=f"po{h}")
                        for h in range(NQ)
                    ]
                    for q in range(NQ):
                        nc.tensor.matmul(
                            po[q][:],
                            ones64[:],
                            b2e[:, q * QW : (q + 1) * QW],
                            start=True, stop=False,
                            skip_group_check=True,
                        )
                    for hb in range(8):
                        for jc in range(4):
                            nc.tensor.matmul(
                                pf[:, hb, :],
                                w1gT[:, jc, hb * 128 : (hb + 1) * 128],
                                zT[:, jc, :],
                                start=False, stop=(jc == 3 and hb == 7),
                                skip_group_check=True,
                            )
                    gmT = epi.tile([128, 8, ST], bf16, tag="gmT")
                    nc.scalar.activation(gmT[:, 0:4, :], pf[:, 0:4, :], AF.Gelu)
                    nc.scalar.activation(gmT[:, 4:8, :], pf[:, 4:8, :], AF.Gelu)

                    # ff2 + residual, in column halves so the first half's
                    # add+store overlaps the second half's matmuls
                    fin = epi.tile([ST, C], f32, tag="fin")
                    for q in range(NQ):
                        cs = slice(q * QW, (q + 1) * QW)
                        for k in range(8):
                            nc.tensor.matmul(
                                po[q][:], gmT[:, k, :], w2T[:, k, cs],
                                start=False, stop=(k == 7),
                                skip_group_check=True,
                            )
                        nc.vector.tensor_add(fin[:, cs], h_sb[:, cs], po[q][:])
                        nc.sync.dma_start(outp[:, cs], fin[:, cs])

    if split:
        _split_multi_waits(nc)
    return nc


def _host_prep(inputs, nj_up=NJ_UP):
    import ml_dtypes

    bf = ml_dtypes.bfloat16
    feat = np.asarray(inputs["feat"], dtype=np.float32)
    batch_idx = np.asarray(inputs["batch_idx"]).astype(np.int64)
    B = int(np.asarray(inputs["batch_size"]))
    query = np.asarray(inputs["query"], dtype=np.float32)
    g_q = np.asarray(inputs["g_q"], np.float32)
    b_q = np.asarray(inputs["b_q"], np.float32)
    w_q = np.asarray(inputs["w_q"], np.float32)
    w_k = np.asarray(inputs["w_k"], np.float32)
    w_v = np.asarray(inputs["w_v"], np.float32)
    b_q_in = np.asarray(inputs["b_q_in"], np.float32)
    b_v_in = np.asarray(inputs["b_v_in"], np.float32)
    w_o = np.asarray(inputs["w_o"], np.float32)
    b_o = np.asarray(inputs["b_o"], np.float32)
    g_ff = np.asarray(inputs["g_ff"], np.float32)
    b_ff = np.asarray(inputs["b_ff"], np.float32)
    w1 = np.asarray(inputs["w1"], np.float32)
    b1 = np.asarray(inputs["b1"], np.float32)
    w2 = np.asarray(inputs["w2"], np.float32)
    b2 = np.asarray(inputs["b2"], np.float32)

    S = B // NCORES
    counts = np.bincount(batch_idx, minlength=B)
    offs = np.concatenate([[0], np.cumsum(counts)])
    NT = max(1, int(np.ceil(counts.max() / 128)))
    P = NT * 128

    # padded per-scene feat in bf16, [B, P, C]
    featpad = np.zeros((B, P, C), dtype=bf)
    for b in range(B):
        featpad[b, : counts[b]] = feat[offs[b] : offs[b + 1]].astype(bf)

    # n-major tiles: per tile [128, C] with rows p = n-within-tile; host
    # layout groups rows as (tile, p) -> flat (S*NT*128, C)
    # SBUF chunk [p, i, c] <- rows (p*cg + i) of the chunk block, so build
    # [tile, 128, C] then per-chunk transpose later.  DMA reads contiguous
    # rows; we lay the whole buffer chunk-by-chunk in [p, i, c] order.
    chunk_lists = [_chunk_plan(NT, ramp=(s == 0)) for s in range(S)]

    featA = np.empty((NCORES, S * NT * 128, C), dtype=bf)
    featTA = (
        np.empty((NCORES, S * NT * 128 * nj_up, 128), dtype=bf) if nj_up else None
    )
    npad = np.empty((NCORES, 128, S), dtype=np.float32)
    for c in range(NCORES):
        for s in range(S):
            b = c * S + s
            npad[c, :, s] = P - counts[b]
            tiles = featpad[b].reshape(NT, 128, C)
            i = 0
            for cg in chunk_lists[s]:
                blk = tiles[i : i + cg]  # [cg, 128, C]
                r0 = (s * NT + i) * 128
                featA[c, r0 : r0 + 128 * cg] = (
                    blk.transpose(1, 0, 2).reshape(128 * cg, C)
                )
                if nj_up:
                    # [p, i, j, n] where channel = j*128 + p
                    tb = blk.reshape(cg, 128, 4, 128)[:, :, :nj_up, :]
                    # tb[i, n, j, p] -> [p, i, j, n]
                    tT = np.ascontiguousarray(tb.transpose(3, 0, 2, 1))
                    rT0 = (s * NT + i) * 128 * nj_up
                    featTA[c, rT0 : rT0 + 128 * cg * nj_up] = tT.reshape(
                        128 * cg * nj_up, 128
                    )
                i += cg

    # query-side fold (host; tiny)
    q = query[0]
    mu = q.mean(-1, keepdims=True)
    var = ((q - mu) ** 2).mean(-1, keepdims=True)
    qn = (q - mu) / np.sqrt(var + 1e-5) * g_q + b_q
    qh = (qn @ w_q.T + b_q_in) / np.sqrt(DH)  # [T, C]
    A = np.einsum(
        "thd,hdc->cht", qh.reshape(T, H, DH), w_k.reshape(H, DH, C)
    ).reshape(C, H * T)

    consts = dict(
        akT=np.ascontiguousarray(A.astype(bf)),
        wvT=np.ascontiguousarray(w_v.T.astype(bf)),
        woT=np.ascontiguousarray(w_o.T.astype(bf)),
        w1gT=np.ascontiguousarray((w1 * g_ff[None, :]).T.astype(bf)),
        b1e=(b1 + w1 @ b_ff).reshape(1, 2 * C).astype(bf),
        w2T=np.ascontiguousarray(w2.T.astype(bf)),
        b2e=b2.reshape(1, C).astype(bf),
        qb=np.ascontiguousarray(query[0] + (b_o + w_o @ b_v_in)[None, :]).astype(
            bf
        ),
        identb=np.eye(128, dtype=bf),
    )
    in_maps = []
    for c in range(NCORES):
        m = dict(consts)
        m["featp"] = featA[c]
        if nj_up:
            m["featTp"] = featTA[c]
        m["npad"] = npad[c]
        in_maps.append(m)
    return in_maps, NT, S, B


def kernel(**inputs):
    from concourse.bass_utils import run_bass_kernel_spmd

    in_maps, NT, S, B = _host_prep(inputs, NJ_UP)
    key = (NT, S, NJ_UP)
    if key not in _CACHE:
        _CACHE[key] = _build(NT, S, nj_up=NJ_UP)
    nc = _CACHE[key]
    res = run_bass_kernel_spmd(nc, in_maps, core_ids=list(range(NCORES)))
    out = np.empty((B, T, C), dtype=np.float32)
    for c in range(NCORES):
        o = res.results[c]["outp"]
        for s in range(S):
            out[c * S + s] = o[s * T : (s + 1) * T]
    return out
